# revision 31
# baseline (speedup 1.0000x reference)
"""AWBNet (wo R2) Trainium2 kernel — v5 (interleaved band layout).

Math (per sample b):
  m = reshape(relu(hist_flat @ W1 + b1) @ W2 + b2, [9, 3])
  feats(px) = [r, g, b, r^2, g^2, b^2, rg, rb, gb]
  y[px, c] = sum_k feats[px, k] * m[k, c]

Device strategy (8 cores, data parallel over batch, 2 samples/core):
  * SBUF partitions hold (band i, group g): 3 bands x 42 pixel groups = 126
    partitions; groups 0..20 belong to sample 0, 21..41 to sample 1. The
    host ships x as xI[42i+g, n] = x_i(pixel (g, n)) plus a band-rotated
    copy xR (pure layout duplication) so rg/gb/br products are
    lane-aligned.
  * Per-pixel combine is THREE block-diagonal matmuls per column chunk:
    out[42c+g, n] += sum_i M_mat[i, c; s(g)] * plane_mat[42i+g, n] for
    plane in {xI (linear), xI^2 (squares, one DVE op), xI*xR (crosses,
    one DVE op)}. The [3,3] diagonal blocks carry per-sample
    coefficients, built from the MLP output with one masked
    tensor_scalar per (mat, c).
  * Tiny MLP on TensorE (fp16 W1 streamed over two DMA queues); ACT
    evicts PSUM -> fp16 planes; host re-interleaves.
"""

import sys

import numpy as np

for _p in ("/opt/trn_rl_repo",):
    if _p not in sys.path:
        sys.path.insert(0, _p)

import concourse.bacc as bacc
import concourse.mybir as mybir
import concourse.tile as tile
from concourse import bass_utils

# ---- problem constants (hardcoded per contract) ----
N_CORES = 8
B, H, W, C = 16, 512, 512, 3
SPC = B // N_CORES  # samples per core = 2
PX_SAMPLE = H * W  # 262144
P = 128

G_S = 21  # pixel groups per sample
G = SPC * G_S  # 42 groups
NP = 3 * G  # 126 used partitions
XCOLS = 12800  # padded pixels per group (21*12800 >= 262144)
NSTAGE = 7  # psum stages: 6 x 2048 + 1 x 512
STAGE_COLS = (2048, 2048, 2048, 2048, 2048, 2048, 512)

HIST = 3 * 64 * 64  # 12288
HID = 256
MOUT = 27
KT = HIST // P  # 96 k-tiles
MT = HID // P  # 2 m-tiles
W1_CH = 8  # k-tiles per W1 DMA chunk

F16 = mybir.dt.float16
F32 = mybir.dt.float32
MULT = mybir.AluOpType.mult

_CACHE = {}


def _colmap(mat, i, c):
    """W2/b2 column for (matrix, band, out-channel): which of the 27
    m-coefficients scales plane_mat band i into channel c."""
    if mat == 0:  # linear: x_i
        k = i
    elif mat == 1:  # squares: x_i^2
        k = 3 + i
    else:  # crosses: x_i * x_{(i+1)%3} -> rg, gb, br
        k = (6, 8, 7)[i]
    return 3 * k + c


def _build():
    nc = bacc.Bacc(
        "TRN2", target_bir_lowering=False, debug=False, num_devices=N_CORES
    )

    xi_d = nc.dram_tensor("xi", [NP, XCOLS], F16, kind="ExternalInput")
    xr_d = nc.dram_tensor("xr", [NP, XCOLS], F16, kind="ExternalInput")
    w1_d = nc.dram_tensor("w1pm", [P, KT, HID], F16, kind="ExternalInput")
    hp_d = nc.dram_tensor("h_packed", [P, KT * SPC], F16, kind="ExternalInput")
    b1_d = nc.dram_tensor("b1_rep", [SPC, HID], F32, kind="ExternalInput")
    w2_d = nc.dram_tensor("w2i", [MT, P, 3 * 9], F32, kind="ExternalInput")
    b2_d = nc.dram_tensor("b2i", [NP, 9], F32, kind="ExternalInput")
    e3_d = nc.dram_tensor("e3", [SPC, 3, NP], F32, kind="ExternalInput")
    mask_d = nc.dram_tensor("maskS", [NP, G], F16, kind="ExternalInput")
    y_d = nc.dram_tensor("y_bands", [NP, XCOLS], F16, kind="ExternalOutput")

    with tile.TileContext(nc) as tc:
        with (
            tc.tile_pool(name="mlp", bufs=1) as mlp_pool,
            tc.tile_pool(name="w1s", bufs=1) as w1_pool,
            tc.tile_pool(name="px", bufs=1) as px_pool,
            tc.tile_pool(name="ring", bufs=2) as ring_pool,
        ):
            # ---------------- input DMAs ----------------
            hp_sb = mlp_pool.tile([P, KT * SPC], F16, tag="hp", name="hp")
            nc.gpsimd.dma_start(out=hp_sb, in_=hp_d[:, :])

            # W1 stream has strict priority: 12 chunks alternate sync/scalar
            # in k-order so arrival order matches MM consumption.
            NCHW1 = KT // W1_CH  # 12
            w1_sbs = []
            for kc in range(NCHW1):
                w1_sb = w1_pool.tile(
                    [P, W1_CH, HID], F16, tag=f"w1c{kc}", name=f"w1c{kc}"
                )
                q = nc.sync if kc % 2 == 0 else nc.scalar
                q.dma_start(out=w1_sb, in_=w1_d[:, kc * W1_CH : (kc + 1) * W1_CH, :])
                w1_sbs.append(w1_sb)

            # x / xR stream per storm stage (xI on sync, xR on scalar), so
            # the storm chases the DMA with stage granularity.
            xi_sb = px_pool.tile([NP, XCOLS], F16, tag="xi", name="xi")
            xr_sb = px_pool.tile([NP, XCOLS], F16, tag="xr", name="xr")
            col0 = 0
            for st in range(NSTAGE):
                sl = slice(col0, col0 + STAGE_COLS[st])
                nc.sync.dma_start(out=xi_sb[:, sl], in_=xi_d[:, sl])
                nc.scalar.dma_start(out=xr_sb[:, sl], in_=xr_d[:, sl])
                col0 += STAGE_COLS[st]

            # small setup DMAs on SWDGE
            b1_sb = mlp_pool.tile([SPC, HID], F32, tag="b1", name="b1")
            nc.gpsimd.dma_start(out=b1_sb, in_=b1_d[:, :])
            w2_sb = mlp_pool.tile([P, MT, 3 * 9], F32, tag="w2", name="w2")
            nc.gpsimd.dma_start(out=w2_sb, in_=w2_d.rearrange("m p n -> p m n"))
            b2_sb = mlp_pool.tile([NP, 9], F32, tag="b2", name="b2")
            nc.gpsimd.dma_start(out=b2_sb, in_=b2_d[:, :])
            e3_sb = mlp_pool.tile([SPC, 3, NP], F32, tag="e3", name="e3")
            nc.gpsimd.dma_start(out=e3_sb, in_=e3_d[:, :, :])
            mask_sb = mlp_pool.tile([NP, G], F16, tag="mask", name="mask")
            nc.gpsimd.dma_start(out=mask_sb, in_=mask_d[:, :])

            # ---------------- MLP (TensorE) ----------------
            with tc.tile_pool(name="mlpps", bufs=1, space="PSUM") as mlp_psum:
                feat_ps = mlp_psum.tile([SPC, HID], F32, tag="featps", name="featps")
                for kc in range(NCHW1):
                    w1_sb = w1_sbs[kc]
                    for kk in range(W1_CH):
                        k = kc * W1_CH + kk
                        nc.tensor.matmul(
                            feat_ps,
                            hp_sb[:, k * SPC : (k + 1) * SPC],
                            w1_sb[:, kk, :],
                            start=(k == 0),
                            stop=(k == KT - 1),
                        )

                feat_sb = mlp_pool.tile([SPC, HID], F32, tag="featsb", name="featsb")
                nc.vector.tensor_add(feat_sb, feat_ps, b1_sb)
                feat_r = mlp_pool.tile([SPC, HID], F32, tag="featr", name="featr")
                nc.vector.tensor_scalar(
                    feat_r, feat_sb, 0.0, None, mybir.AluOpType.max
                )

                # msP9[42i+21s+g', 3*mat+c] = band-selected m-coefficients.
                # patt_i = feat_slice^T @ E_i (E_i = 0/1 selector, zero
                # outside band i); all six into one PSUM tile, one eviction,
                # then band matmuls accumulate over (mt, i).
                pt_ps = mlp_psum.tile([P, 2 * 3, P], F32, tag="ptps", name="ptps")
                for mt in range(MT):
                    for i in range(3):
                        nc.tensor.matmul(
                            pt_ps[:, 3 * mt + i, 0:NP],
                            feat_r[:, mt * P : (mt + 1) * P],
                            e3_sb[:, i, :],
                            start=True,
                            stop=True,
                        )
                patt = mlp_pool.tile([P, 2 * 3, P], F32, tag="patt", name="patt")
                nc.vector.tensor_copy(patt, pt_ps)
                msP_ps = mlp_psum.tile([NP, 9], F32, tag="msps", name="msps")
                nmm = 0
                for mt in range(MT):
                    for i in range(3):
                        nc.tensor.matmul(
                            msP_ps,
                            patt[:, 3 * mt + i, 0:NP],
                            w2_sb[:, mt, 9 * i : 9 * (i + 1)],
                            start=(nmm == 0),
                            stop=(nmm == 2 * MT + 1),
                        )
                        nmm += 1
                msP9 = mlp_pool.tile([NP, 9], F32, tag="msP9", name="msP9")
                nc.vector.tensor_add(msP9, msP_ps, b2_sb)

            # block-diagonal weight matrices: one masked tensor_scalar per
            # (mat, c) writes column band c of lhsT_mat (full width)
            lhsT = mlp_pool.tile([NP, 3, NP], F16, tag="lhsT", name="lhsT")
            for mat in range(3):
                for c in range(C):
                    nc.vector.tensor_scalar(
                        lhsT[:, mat, G * c : G * (c + 1)],
                        mask_sb,
                        msP9[:, 3 * mat + c : 3 * mat + c + 1],
                        None,
                        MULT,
                    )

            # ---------------- storm: per-stage basis + 3 matmuls ----------
            with tc.tile_pool(name="pxps", bufs=2, space="PSUM") as px_psum:
                col0 = 0
                for st in range(NSTAGE):
                    ncols = STAGE_COLS[st]
                    sl = slice(col0, col0 + ncols)

                    sq_t = ring_pool.tile([NP, 2048], F16, tag="sq", name=f"sq{st}")
                    nc.vector.tensor_mul(
                        sq_t[:, 0:ncols], xi_sb[:, sl], xi_sb[:, sl]
                    )
                    cr_t = ring_pool.tile([NP, 2048], F16, tag="cr", name=f"cr{st}")
                    nc.vector.tensor_mul(
                        cr_t[:, 0:ncols], xi_sb[:, sl], xr_sb[:, sl]
                    )

                    yc_ps = px_psum.tile([NP, 2048], F32, tag="yc", name=f"yc{st}")
                    nch = (ncols + 511) // 512
                    for mat in range(3):
                        for n in range(nch):
                            c0 = n * 512
                            c1 = min(c0 + 512, ncols)
                            if mat == 0:
                                rhs = xi_sb[:, col0 + c0 : col0 + c1]
                            elif mat == 1:
                                rhs = sq_t[:, c0:c1]
                            else:
                                rhs = cr_t[:, c0:c1]
                            nc.tensor.matmul(
                                yc_ps[:, c0:c1],
                                lhsT[:, mat, :],
                                rhs,
                                start=(mat == 0),
                                stop=(mat == 2),
                            )

                    y_sb = ring_pool.tile([NP, 2048], F16, tag="ysb", name=f"y{st}")
                    nc.scalar.copy(y_sb[:, 0:ncols], yc_ps[:, 0:ncols])
                    yq = nc.sync if st % 2 == 0 else nc.scalar
                    yq.dma_start(out=y_d[:, sl], in_=y_sb[:, 0:ncols])
                    col0 += ncols

    nc.compile()
    return nc


def _prep_inputs(x, histogram, W1, b1, W2, b2):
    """Host-side sharding / layout packing (layout + dtype only; no data
    arithmetic)."""
    x = np.asarray(x, dtype=np.float32)
    hist = np.asarray(histogram, dtype=np.float32).reshape(B, HIST)
    W1 = np.asarray(W1, dtype=np.float32)
    b1 = np.asarray(b1, dtype=np.float32)
    W2 = np.asarray(W2, dtype=np.float32)
    b2 = np.asarray(b2, dtype=np.float32)

    w1pm = np.ascontiguousarray(
        W1.reshape(KT, P, HID).transpose(1, 0, 2)
    ).astype(np.float16)
    b1rep = np.ascontiguousarray(np.broadcast_to(b1, (SPC, HID)))
    e3 = np.zeros((SPC, 3, NP), dtype=np.float32)
    for i in range(3):
        for s in range(SPC):
            e3[s, i, 42 * i + G_S * s : 42 * i + G_S * (s + 1)] = 1.0

    # W2 / b2 with interleave-mapped columns
    cm = np.empty((3, 9), dtype=np.int64)  # [i, 3*mat+c]
    for i in range(3):
        for mat in range(3):
            for c in range(C):
                cm[i, 3 * mat + c] = _colmap(mat, i, c)
    w2i = np.ascontiguousarray(
        W2.reshape(MT, P, MOUT)[:, :, cm.reshape(-1)].reshape(MT, P, 3, 9)
        .reshape(MT, P, 27)
    )
    b2i = np.empty((NP, 9), dtype=np.float32)
    for i in range(3):
        b2i[42 * i : 42 * (i + 1), :] = b2[cm[i]]

    maskS = np.zeros((NP, G), dtype=np.float16)
    for i in range(3):
        for g in range(G):
            maskS[42 * i + g, g] = 1.0

    rot = np.concatenate(
        [np.arange(42, 84), np.arange(84, 126), np.arange(0, 42)]
    )

    in_maps = []
    for core in range(N_CORES):
        xI = np.zeros((NP, XCOLS), dtype=np.float16)
        for s in range(SPC):
            xs = x[core * SPC + s].reshape(PX_SAMPLE, C)
            pad = np.zeros((G_S * XCOLS, C), dtype=np.float32)
            pad[:PX_SAMPLE] = xs
            v = pad.reshape(G_S, XCOLS, C)  # [g', n, i]
            for i in range(3):
                xI[42 * i + G_S * s : 42 * i + G_S * (s + 1), :] = v[:, :, i].astype(
                    np.float16
                )
        xR = np.ascontiguousarray(xI[rot])

        h_core = hist[core * SPC : (core + 1) * SPC]
        hp = np.ascontiguousarray(
            h_core.reshape(SPC, KT, P).transpose(2, 1, 0).reshape(P, KT * SPC)
        ).astype(np.float16)
        in_maps.append(
            {
                "xi": xI,
                "xr": xR,
                "w1pm": w1pm,
                "h_packed": hp,
                "b1_rep": b1rep,
                "w2i": w2i,
                "b2i": b2i,
                "e3": e3,
                "maskS": maskS,
            }
        )
    return in_maps


def _unpack_output(res):
    y = np.empty((B, H, W, C), dtype=np.float32)
    for core in range(N_CORES):
        yb = np.asarray(res.results[core]["y_bands"])  # [126, XCOLS] f16
        for s in range(SPC):
            v = yb[:, :].reshape(3, G, XCOLS)[:, G_S * s : G_S * (s + 1), :]
            # v[c, g', n] -> pixel g'*XCOLS + n
            flat = v.transpose(1, 2, 0).reshape(G_S * XCOLS, C)[:PX_SAMPLE]
            y[core * SPC + s] = flat.reshape(H, W, C).astype(np.float32)
    return y


def run(trace=False, **inputs):
    if "nc" not in _CACHE:
        _CACHE["nc"] = _build()
    nc = _CACHE["nc"]
    in_maps = _prep_inputs(**inputs)
    res = bass_utils.run_bass_kernel_spmd(
        nc, in_maps, core_ids=list(range(N_CORES)), trace=trace
    )
    y = _unpack_output(res)
    return y, res


def kernel(**inputs) -> np.ndarray:
    y, _ = run(trace=False, **inputs)
    return y


if __name__ == "__main__":
    rng = np.random.default_rng(0)
    ins = {
        "x": rng.random((B, H, W, C), dtype=np.float32),
        "histogram": rng.random((B, 3, 64, 64), dtype=np.float32),
        "W1": (rng.standard_normal((HIST, HID)) / np.sqrt(HIST)).astype(np.float32),
        "b1": np.zeros(HID, np.float32),
        "W2": (rng.standard_normal((HID, MOUT)) / np.sqrt(HID)).astype(np.float32),
        "b2": np.zeros(MOUT, np.float32),
    }
    y = kernel(**ins)
    print("out", y.shape, y.dtype, float(np.abs(y).max()))


# revision 32
# speedup vs baseline: 1.0800x; 1.0800x over previous
"""AWBNet (wo R2) Trainium2 kernel — v5 (interleaved band layout).

Math (per sample b):
  m = reshape(relu(hist_flat @ W1 + b1) @ W2 + b2, [9, 3])
  feats(px) = [r, g, b, r^2, g^2, b^2, rg, rb, gb]
  y[px, c] = sum_k feats[px, k] * m[k, c]

Device strategy (8 cores, data parallel over batch, 2 samples/core):
  * SBUF partitions hold (band i, group g): 3 bands x 42 pixel groups = 126
    partitions; groups 0..20 belong to sample 0, 21..41 to sample 1. The
    host ships x as xI[42i+g, n] = x_i(pixel (g, n)) plus a band-rotated
    copy xR (pure layout duplication) so rg/gb/br products are
    lane-aligned.
  * Per-pixel combine is THREE block-diagonal matmuls per column chunk:
    out[42c+g, n] += sum_i M_mat[i, c; s(g)] * plane_mat[42i+g, n] for
    plane in {xI (linear), xI^2 (squares, one DVE op), xI*xR (crosses,
    one DVE op)}. The [3,3] diagonal blocks carry per-sample
    coefficients, built from the MLP output with one masked
    tensor_scalar per (mat, c).
  * Tiny MLP on TensorE (fp16 W1 streamed over two DMA queues); ACT
    evicts PSUM -> fp16 planes; host re-interleaves.
"""

import sys

import numpy as np

for _p in ("/opt/trn_rl_repo",):
    if _p not in sys.path:
        sys.path.insert(0, _p)

import concourse.bacc as bacc
import concourse.mybir as mybir
import concourse.tile as tile
from concourse import bass_utils

# ---- problem constants (hardcoded per contract) ----
N_CORES = 8
B, H, W, C = 16, 512, 512, 3
SPC = B // N_CORES  # samples per core = 2
PX_SAMPLE = H * W  # 262144
P = 128

G_S = 21  # pixel groups per sample
G = SPC * G_S  # 42 groups
NP = 3 * G  # 126 used partitions
XCOLS = 12800  # padded pixels per group (21*12800 >= 262144)
NSTAGE = 7  # psum stages: 6 x 2048 + 1 x 512
STAGE_COLS = (2048, 2048, 2048, 2048, 2048, 2048, 512)

HIST = 3 * 64 * 64  # 12288
HID = 256
MOUT = 27
KT = HIST // P  # 96 k-tiles
MT = HID // P  # 2 m-tiles
W1_CH = 8  # k-tiles per W1 DMA chunk

F16 = mybir.dt.float16
F32 = mybir.dt.float32
MULT = mybir.AluOpType.mult

_CACHE = {}


def _colmap(mat, i, c):
    """W2/b2 column for (matrix, band, out-channel): which of the 27
    m-coefficients scales plane_mat band i into channel c."""
    if mat == 0:  # linear: x_i
        k = i
    elif mat == 1:  # squares: x_i^2
        k = 3 + i
    else:  # crosses: x_i * x_{(i+1)%3} -> rg, gb, br
        k = (6, 8, 7)[i]
    return 3 * k + c


def _build():
    nc = bacc.Bacc(
        "TRN2", target_bir_lowering=False, debug=False, num_devices=N_CORES
    )

    xi_d = nc.dram_tensor("xi", [NP, XCOLS], F16, kind="ExternalInput")
    xr_d = nc.dram_tensor("xr", [NP, XCOLS], F16, kind="ExternalInput")
    w1_d = nc.dram_tensor("w1pm", [P, KT, HID], F16, kind="ExternalInput")
    hp_d = nc.dram_tensor("h_packed", [P, KT * SPC], F16, kind="ExternalInput")
    b1_d = nc.dram_tensor("b1_rep", [SPC, HID], F32, kind="ExternalInput")
    w2_d = nc.dram_tensor("w2i", [MT, P, 3 * 9], F32, kind="ExternalInput")
    b2_d = nc.dram_tensor("b2i", [NP, 9], F32, kind="ExternalInput")
    e3_d = nc.dram_tensor("e3", [SPC, 3, NP], F32, kind="ExternalInput")
    mask_d = nc.dram_tensor("maskS", [NP, G], F16, kind="ExternalInput")
    y_d = nc.dram_tensor("y_bands", [NP, XCOLS], F16, kind="ExternalOutput")

    with tile.TileContext(nc) as tc:
        with (
            tc.tile_pool(name="mlp", bufs=1) as mlp_pool,
            tc.tile_pool(name="w1s", bufs=1) as w1_pool,
            tc.tile_pool(name="px", bufs=1) as px_pool,
            tc.tile_pool(name="ring", bufs=2) as ring_pool,
        ):
            # ---------------- input DMAs ----------------
            hp_sb = mlp_pool.tile([P, KT * SPC], F16, tag="hp", name="hp")
            nc.gpsimd.dma_start(out=hp_sb, in_=hp_d[:, :])

            # W1 stream has strict priority: 12 chunks alternate sync/scalar
            # in k-order so arrival order matches MM consumption.
            NCHW1 = KT // W1_CH  # 12
            w1_sbs = []
            for kc in range(NCHW1):
                w1_sb = w1_pool.tile(
                    [P, W1_CH, HID], F16, tag=f"w1c{kc}", name=f"w1c{kc}"
                )
                q = nc.sync if kc % 2 == 0 else nc.scalar
                q.dma_start(out=w1_sb, in_=w1_d[:, kc * W1_CH : (kc + 1) * W1_CH, :])
                w1_sbs.append(w1_sb)

            # x streams per storm stage on sync. The band-rotated factor xR:
            # stages 0-3 are SBUF->SBUF copies on the idle SWDGE queue
            # (chasing xI, zero HBM cost); stages 4-6 come from HBM on
            # scalar (which is otherwise free after its W1 half).
            xi_sb = px_pool.tile([NP, XCOLS], F16, tag="xi", name="xi")
            xr_sb = px_pool.tile([NP, XCOLS], F16, tag="xr", name="xr")
            col0 = 0
            for st in range(NSTAGE):
                sl = slice(col0, col0 + STAGE_COLS[st])
                nc.sync.dma_start(out=xi_sb[:, sl], in_=xi_d[:, sl])
                if st >= 4:
                    nc.scalar.dma_start(out=xr_sb[:, sl], in_=xr_d[:, sl])
                col0 += STAGE_COLS[st]
            col0 = 0
            for st in range(NSTAGE):
                sl = slice(col0, col0 + STAGE_COLS[st])
                if st < 4:
                    nc.gpsimd.dma_start(
                        out=xr_sb[0:84, sl], in_=xi_sb[42:126, sl]
                    )
                    nc.gpsimd.dma_start(
                        out=xr_sb[84:126, sl], in_=xi_sb[0:42, sl]
                    )
                col0 += STAGE_COLS[st]

            # small setup DMAs on SWDGE
            b1_sb = mlp_pool.tile([SPC, HID], F32, tag="b1", name="b1")
            nc.gpsimd.dma_start(out=b1_sb, in_=b1_d[:, :])
            w2_sb = mlp_pool.tile([P, MT, 3 * 9], F32, tag="w2", name="w2")
            nc.gpsimd.dma_start(out=w2_sb, in_=w2_d.rearrange("m p n -> p m n"))
            b2_sb = mlp_pool.tile([NP, 9], F32, tag="b2", name="b2")
            nc.gpsimd.dma_start(out=b2_sb, in_=b2_d[:, :])
            e3_sb = mlp_pool.tile([SPC, 3, NP], F32, tag="e3", name="e3")
            nc.gpsimd.dma_start(out=e3_sb, in_=e3_d[:, :, :])
            mask_sb = mlp_pool.tile([NP, G], F16, tag="mask", name="mask")
            nc.gpsimd.dma_start(out=mask_sb, in_=mask_d[:, :])

            # ---------------- MLP (TensorE) ----------------
            with tc.tile_pool(name="mlpps", bufs=1, space="PSUM") as mlp_psum:
                feat_ps = mlp_psum.tile([SPC, HID], F32, tag="featps", name="featps")
                for kc in range(NCHW1):
                    w1_sb = w1_sbs[kc]
                    for kk in range(W1_CH):
                        k = kc * W1_CH + kk
                        nc.tensor.matmul(
                            feat_ps,
                            hp_sb[:, k * SPC : (k + 1) * SPC],
                            w1_sb[:, kk, :],
                            start=(k == 0),
                            stop=(k == KT - 1),
                        )

                feat_sb = mlp_pool.tile([SPC, HID], F32, tag="featsb", name="featsb")
                nc.vector.tensor_add(feat_sb, feat_ps, b1_sb)
                feat_r = mlp_pool.tile([SPC, HID], F32, tag="featr", name="featr")
                nc.vector.tensor_scalar(
                    feat_r, feat_sb, 0.0, None, mybir.AluOpType.max
                )

                # msP9[42i+21s+g', 3*mat+c] = band-selected m-coefficients.
                # patt_i = feat_slice^T @ E_i (E_i = 0/1 selector, zero
                # outside band i); all six into one PSUM tile, one eviction,
                # then band matmuls accumulate over (mt, i).
                pt_ps = mlp_psum.tile([P, 2 * 3, P], F32, tag="ptps", name="ptps")
                for mt in range(MT):
                    for i in range(3):
                        nc.tensor.matmul(
                            pt_ps[:, 3 * mt + i, 0:NP],
                            feat_r[:, mt * P : (mt + 1) * P],
                            e3_sb[:, i, :],
                            start=True,
                            stop=True,
                        )
                patt = mlp_pool.tile([P, 2 * 3, P], F32, tag="patt", name="patt")
                nc.vector.tensor_copy(patt, pt_ps)
                msP_ps = mlp_psum.tile([NP, 9], F32, tag="msps", name="msps")
                nmm = 0
                for mt in range(MT):
                    for i in range(3):
                        nc.tensor.matmul(
                            msP_ps,
                            patt[:, 3 * mt + i, 0:NP],
                            w2_sb[:, mt, 9 * i : 9 * (i + 1)],
                            start=(nmm == 0),
                            stop=(nmm == 2 * MT + 1),
                        )
                        nmm += 1
                msP9 = mlp_pool.tile([NP, 9], F32, tag="msP9", name="msP9")
                nc.vector.tensor_add(msP9, msP_ps, b2_sb)

            # block-diagonal weight matrices: one masked tensor_scalar per
            # (mat, c) writes column band c of lhsT_mat (full width)
            lhsT = mlp_pool.tile([NP, 3, NP], F16, tag="lhsT", name="lhsT")
            for mat in range(3):
                for c in range(C):
                    nc.vector.tensor_scalar(
                        lhsT[:, mat, G * c : G * (c + 1)],
                        mask_sb,
                        msP9[:, 3 * mat + c : 3 * mat + c + 1],
                        None,
                        MULT,
                    )

            # ---------------- storm: per-stage basis + 3 matmuls ----------
            with tc.tile_pool(name="pxps", bufs=2, space="PSUM") as px_psum:
                col0 = 0
                for st in range(NSTAGE):
                    ncols = STAGE_COLS[st]
                    sl = slice(col0, col0 + ncols)

                    sq_t = ring_pool.tile([NP, 2048], F16, tag="sq", name=f"sq{st}")
                    nc.vector.tensor_mul(
                        sq_t[:, 0:ncols], xi_sb[:, sl], xi_sb[:, sl]
                    )
                    cr_t = ring_pool.tile([NP, 2048], F16, tag="cr", name=f"cr{st}")
                    nc.vector.tensor_mul(
                        cr_t[:, 0:ncols], xi_sb[:, sl], xr_sb[:, sl]
                    )

                    yc_ps = px_psum.tile([NP, 2048], F32, tag="yc", name=f"yc{st}")
                    nch = (ncols + 511) // 512
                    for mat in range(3):
                        for n in range(nch):
                            c0 = n * 512
                            c1 = min(c0 + 512, ncols)
                            if mat == 0:
                                rhs = xi_sb[:, col0 + c0 : col0 + c1]
                            elif mat == 1:
                                rhs = sq_t[:, c0:c1]
                            else:
                                rhs = cr_t[:, c0:c1]
                            nc.tensor.matmul(
                                yc_ps[:, c0:c1],
                                lhsT[:, mat, :],
                                rhs,
                                start=(mat == 0),
                                stop=(mat == 2),
                            )

                    y_sb = ring_pool.tile([NP, 2048], F16, tag="ysb", name=f"y{st}")
                    nc.scalar.copy(y_sb[:, 0:ncols], yc_ps[:, 0:ncols])
                    yq = nc.sync if st % 2 == 0 else nc.scalar
                    yq.dma_start(out=y_d[:, sl], in_=y_sb[:, 0:ncols])
                    col0 += ncols

    nc.compile()
    return nc


def _prep_inputs(x, histogram, W1, b1, W2, b2):
    """Host-side sharding / layout packing (layout + dtype only; no data
    arithmetic)."""
    x = np.asarray(x, dtype=np.float32)
    hist = np.asarray(histogram, dtype=np.float32).reshape(B, HIST)
    W1 = np.asarray(W1, dtype=np.float32)
    b1 = np.asarray(b1, dtype=np.float32)
    W2 = np.asarray(W2, dtype=np.float32)
    b2 = np.asarray(b2, dtype=np.float32)

    w1pm = np.ascontiguousarray(
        W1.reshape(KT, P, HID).transpose(1, 0, 2)
    ).astype(np.float16)
    b1rep = np.ascontiguousarray(np.broadcast_to(b1, (SPC, HID)))
    e3 = np.zeros((SPC, 3, NP), dtype=np.float32)
    for i in range(3):
        for s in range(SPC):
            e3[s, i, 42 * i + G_S * s : 42 * i + G_S * (s + 1)] = 1.0

    # W2 / b2 with interleave-mapped columns
    cm = np.empty((3, 9), dtype=np.int64)  # [i, 3*mat+c]
    for i in range(3):
        for mat in range(3):
            for c in range(C):
                cm[i, 3 * mat + c] = _colmap(mat, i, c)
    w2i = np.ascontiguousarray(
        W2.reshape(MT, P, MOUT)[:, :, cm.reshape(-1)].reshape(MT, P, 3, 9)
        .reshape(MT, P, 27)
    )
    b2i = np.empty((NP, 9), dtype=np.float32)
    for i in range(3):
        b2i[42 * i : 42 * (i + 1), :] = b2[cm[i]]

    maskS = np.zeros((NP, G), dtype=np.float16)
    for i in range(3):
        for g in range(G):
            maskS[42 * i + g, g] = 1.0

    rot = np.concatenate(
        [np.arange(42, 84), np.arange(84, 126), np.arange(0, 42)]
    )

    in_maps = []
    for core in range(N_CORES):
        xI = np.zeros((NP, XCOLS), dtype=np.float16)
        for s in range(SPC):
            xs = x[core * SPC + s].reshape(PX_SAMPLE, C)
            pad = np.zeros((G_S * XCOLS, C), dtype=np.float32)
            pad[:PX_SAMPLE] = xs
            v = pad.reshape(G_S, XCOLS, C)  # [g', n, i]
            for i in range(3):
                xI[42 * i + G_S * s : 42 * i + G_S * (s + 1), :] = v[:, :, i].astype(
                    np.float16
                )
        xR = np.ascontiguousarray(xI[rot])

        h_core = hist[core * SPC : (core + 1) * SPC]
        hp = np.ascontiguousarray(
            h_core.reshape(SPC, KT, P).transpose(2, 1, 0).reshape(P, KT * SPC)
        ).astype(np.float16)
        in_maps.append(
            {
                "xi": xI,
                "xr": xR,
                "w1pm": w1pm,
                "h_packed": hp,
                "b1_rep": b1rep,
                "w2i": w2i,
                "b2i": b2i,
                "e3": e3,
                "maskS": maskS,
            }
        )
    return in_maps


def _unpack_output(res):
    y = np.empty((B, H, W, C), dtype=np.float32)
    for core in range(N_CORES):
        yb = np.asarray(res.results[core]["y_bands"])  # [126, XCOLS] f16
        for s in range(SPC):
            v = yb[:, :].reshape(3, G, XCOLS)[:, G_S * s : G_S * (s + 1), :]
            # v[c, g', n] -> pixel g'*XCOLS + n
            flat = v.transpose(1, 2, 0).reshape(G_S * XCOLS, C)[:PX_SAMPLE]
            y[core * SPC + s] = flat.reshape(H, W, C).astype(np.float32)
    return y


def run(trace=False, **inputs):
    if "nc" not in _CACHE:
        _CACHE["nc"] = _build()
    nc = _CACHE["nc"]
    in_maps = _prep_inputs(**inputs)
    res = bass_utils.run_bass_kernel_spmd(
        nc, in_maps, core_ids=list(range(N_CORES)), trace=trace
    )
    y = _unpack_output(res)
    return y, res


def kernel(**inputs) -> np.ndarray:
    y, _ = run(trace=False, **inputs)
    return y


if __name__ == "__main__":
    rng = np.random.default_rng(0)
    ins = {
        "x": rng.random((B, H, W, C), dtype=np.float32),
        "histogram": rng.random((B, 3, 64, 64), dtype=np.float32),
        "W1": (rng.standard_normal((HIST, HID)) / np.sqrt(HIST)).astype(np.float32),
        "b1": np.zeros(HID, np.float32),
        "W2": (rng.standard_normal((HID, MOUT)) / np.sqrt(HID)).astype(np.float32),
        "b2": np.zeros(MOUT, np.float32),
    }
    y = kernel(**ins)
    print("out", y.shape, y.dtype, float(np.abs(y).max()))
